# revision 1
# baseline (speedup 1.0000x reference)
"""Trainium2 Bass kernel for nn_EstimateGrassmann.

Math: for each sample b with z = 1-x (indicator of zeros),
  m_b = sigma @ diag(2x-1) + diag(1-x)  and  |det(m_b)| = |det(sigma - diag(z))|.
Since p_b = det(m_b) > 0, log p_b = sum_k log|pivot_k| of an unpivoted
Gaussian elimination of mtilde = sigma - diag(z).  mtilde is strongly
diagonally dominant in magnitude (pivots stay in ~[0.46, 0.54]), so no
pivoting is needed.

Layout: 128 samples per partition-tile, the 32x32 matrix flattened along
the free dimension; G tiles are eliminated together so each elimination
step is 4 wide DVE instructions (reciprocal, column scale, broadcast
outer product, subtract) covering G*128 samples via multi-dim access
patterns with stride-0 broadcasts.  log|pivot| = 0.5*Ln(pivot^2): Square
and Ln ride the scalar engine, the per-tile reduction on the vector
engine.

sigma = inv(B_ @ inv(C_) + I) is computed on-device; inverses via Newton
iteration X <- X(2I - AX) on the tensor engine, carrying X^T through the
iteration so no per-step transposes are needed.  stabilize() makes both
matrices strongly row-diagonally dominant, so X0 = alpha*I with a fixed
alpha below the guaranteed 1/maxabsrowsum bound converges quadratically.

Sharding: pure data parallel over the batch (65536/8 = 8192 samples per
core; B, C replicated).  Each core returns [128, NTILES] partial sums of
Ln(pivot^2); the host all-reduces with a float64 sum * 0.5 / BATCH.
"""

import numpy as np

DIM = 32
BATCH = 65536
NCORES = 8
P = 128
SHARD = BATCH // NCORES          # 8192
NTILES_FULL = SHARD // P         # 64
GROUP = 16

ALPHA_C = 1.0 / 4.0
ALPHA_LAM = 1.0 / 5.0
NEWTON_ITERS = 8

_cache = {}


def _build(ntiles, group):
    import concourse.bass as bass
    import concourse.mybir as mybir
    from concourse.tile import TileContext

    fp32 = mybir.dt.float32
    i32 = mybir.dt.int32
    AF = mybir.ActivationFunctionType
    OP = mybir.AluOpType
    AX = mybir.AxisListType

    G = min(group, ntiles)
    assert ntiles % G == 0
    ngroups = ntiles // G
    nshard = ntiles * P
    nc = bass.Bass()
    x_d = nc.dram_tensor("x", [nshard, DIM], i32, kind="ExternalInput")
    b_d = nc.dram_tensor("B", [DIM, DIM], fp32, kind="ExternalInput")
    c_d = nc.dram_tensor("C", [DIM, DIM], fp32, kind="ExternalInput")
    eye_d = nc.dram_tensor("eye", [DIM, DIM], fp32, kind="ExternalInput")
    out_d = nc.dram_tensor("out", [P, ntiles], fp32, kind="ExternalOutput")

    with TileContext(nc) as tc:
        with tc.tile_pool(name="const", bufs=1) as cpool, \
             tc.tile_pool(name="setup", bufs=1) as spool, \
             tc.tile_pool(name="psum", bufs=2, space="PSUM") as qpool, \
             tc.tile_pool(name="dram", bufs=1, space="DRAM") as dpool, \
             tc.tile_pool(name="big", bufs=1) as gpool, \
             tc.tile_pool(name="m", bufs=2 if group <= 8 else 1) as mpool, \
             tc.tile_pool(name="t", bufs=2 if group <= 8 else 1) as tpool, \
             tc.tile_pool(name="cs", bufs=2) as cspool, \
             tc.tile_pool(name="r", bufs=4) as rpool, \
             tc.tile_pool(name="d2", bufs=2) as d2pool:

            eye = cpool.tile([DIM, DIM], fp32, name="eye_sb")
            ome = cpool.tile([DIM, DIM], fp32, name="ome_sb")
            eye2 = cpool.tile([DIM, DIM], fp32, name="eye2_sb")
            nc.sync.dma_start(eye[:], eye_d[:])
            # derive 1-eye and 2*eye on device (fewer DMA sems to wait on)
            nc.vector.tensor_scalar(ome[:], eye[:], -1.0, 1.0,
                                    op0=OP.mult, op1=OP.add)
            nc.vector.tensor_scalar(eye2[:], eye[:], 2.0, None, op0=OP.mult)

            def stabilize(m_sb, nm):
                off = spool.tile([DIM, DIM], fp32, name=f"off_{nm}")
                rl = spool.tile([DIM, DIM], fp32, name=f"rl_{nm}")
                ab = spool.tile([DIM, DIM], fp32, name=f"ab_{nm}")
                ab2 = spool.tile([DIM, DIM], fp32, name=f"ab2_{nm}")
                rs = spool.tile([DIM, 1], fp32, name=f"rs_{nm}")
                rs2 = spool.tile([DIM, 1], fp32, name=f"rs2_{nm}")
                st = spool.tile([DIM, DIM], fp32, name=f"st_{nm}")
                nc.vector.tensor_mul(off[:], m_sb[:], ome[:])
                nc.scalar.activation(rl[:], m_sb[:], AF.Relu)
                # diag of stabilized matrix = sum_j |off_ij| + relu(M_ii)
                nc.scalar.activation(ab[:], off[:], AF.Abs)
                nc.vector.tensor_mul(ab2[:], rl[:], eye[:])
                nc.vector.tensor_reduce(rs[:], ab[:], axis=AX.X, op=OP.add)
                nc.vector.tensor_reduce(rs2[:], ab2[:], axis=AX.X, op=OP.add)
                nc.vector.tensor_add(rs[:], rs[:], rs2[:])
                nc.vector.scalar_tensor_tensor(
                    st[:], eye[:], rs[:, 0:1], off[:], op0=OP.mult, op1=OP.add)
                return st

            def transpose32(src, nm):
                ps = qpool.tile([DIM, DIM], fp32, name=f"pt_{nm}", tag="ps")
                dst = spool.tile([DIM, DIM], fp32, name=f"tr_{nm}")
                nc.tensor.transpose(ps[:], src[:], eye[:])
                nc.scalar.copy(dst[:], ps[:])
                return dst

            def inv32(a_sb, alpha, nm):
                at = transpose32(a_sb, nm)
                xx = spool.tile([DIM, DIM], fp32, name=f"x_{nm}")
                xt = spool.tile([DIM, DIM], fp32, name=f"xt_{nm}")
                nc.vector.tensor_scalar(xx[:], eye[:], alpha, None, op0=OP.mult)
                nc.vector.tensor_scalar(xt[:], eye[:], alpha, None, op0=OP.mult)
                for it in range(NEWTON_ITERS):
                    t1 = qpool.tile([DIM, DIM], fp32, name=f"nt_{nm}_{it}", tag="ps")
                    nc.tensor.matmul(t1[:], at[:], xx[:], start=True, stop=True)
                    w = spool.tile([DIM, DIM], fp32, name=f"w_{nm}_{it}", tag="w")
                    nc.vector.scalar_tensor_tensor(
                        w[:], t1[:], -1.0, eye2[:], op0=OP.mult, op1=OP.add)
                    x2 = qpool.tile([DIM, DIM], fp32, name=f"x2_{nm}_{it}", tag="ps2")
                    xt2 = qpool.tile([DIM, DIM], fp32, name=f"xt2_{nm}_{it}", tag="ps3")
                    nc.tensor.matmul(x2[:], xt[:], w[:], start=True, stop=True)
                    nc.tensor.matmul(xt2[:], w[:], xt[:], start=True, stop=True)
                    nc.scalar.copy(xx[:], x2[:])
                    nc.scalar.copy(xt[:], xt2[:])
                return xx, xt

            b_sb = spool.tile([DIM, DIM], fp32, name="b_sb")
            c_sb = spool.tile([DIM, DIM], fp32, name="c_sb")
            nc.sync.dma_start(b_sb[:], b_d[:])
            nc.sync.dma_start(c_sb[:], c_d[:])
            bs = stabilize(b_sb, "b")
            cs = stabilize(c_sb, "c")
            invc, _ = inv32(cs, ALPHA_C, "c")
            bt = transpose32(bs, "bt")
            lamp = qpool.tile([DIM, DIM], fp32, name="lamp", tag="ps")
            nc.tensor.matmul(lamp[:], bt[:], invc[:], start=True, stop=True)
            lam = spool.tile([DIM, DIM], fp32, name="lam")
            nc.vector.tensor_add(lam[:], lamp[:], eye[:])
            sigma, _ = inv32(lam, ALPHA_LAM, "s")

            # replicate sigma to all 128 partitions via a DRAM round trip
            sig_dram = dpool.tile([DIM, DIM], fp32, name="sig_dram")
            nc.sync.dma_start(sig_dram[:], sigma[:])
            sig_rep = gpool.tile([P, DIM * DIM], fp32, name="sig_rep")
            src = sig_dram[:].rearrange("a b -> (a b)").unsqueeze(0) \
                             .broadcast_to([P, DIM * DIM])
            nc.sync.dma_start(sig_rep[:], src)

            # load x as [128, ntiles, 32]; zf = 1 - x (fp32)
            xi = gpool.tile([P, ntiles * DIM], i32, name="xi")
            xv = x_d[:].rearrange("(t p) d -> p t d", p=P)
            nc.sync.dma_start(xi[:].rearrange("p (t d) -> p t d", d=DIM), xv)
            zf = gpool.tile([P, ntiles * DIM], fp32, name="zf")
            nc.vector.tensor_scalar(zf[:], xi[:], -1.0, 1.0,
                                    op0=OP.mult, op1=OP.add)
            zf3 = zf[:].rearrange("p (t d) -> p t d", d=DIM)

            partials = gpool.tile([P, ntiles], fp32, name="partials")
            sig_bc = sig_rep[:].unsqueeze(1).broadcast_to([P, G, DIM * DIM])

            for g in range(ngroups):
                m = mpool.tile([P, G * DIM * DIM], fp32, name=f"m_{g}", tag="m")
                m3 = m.rearrange("p (g f) -> p g f", f=DIM * DIM)
                mv = m.rearrange("p (g i j) -> p g i j", i=DIM, j=DIM)
                nc.gpsimd.tensor_copy(m3, sig_bc)
                dview = m3[:, :, 0:DIM * DIM:DIM + 1]          # [P, G, 32]
                nc.vector.tensor_sub(dview, dview,
                                     zf3[:, g * G:(g + 1) * G, :])
                rg = rpool.tile([P, G], fp32, name=f"r_{g}", tag="r")
                csg = cspool.tile([P, G * DIM], fp32, name=f"cs_{g}", tag="cs")
                cs3 = csg.rearrange("p (g i) -> p g i", g=G)
                for k in range(DIM - 1):
                    n = DIM - 1 - k
                    nc.vector.reciprocal(rg[:], mv[:, :, k, k])
                    col = mv[:, :, k + 1:, k]                  # [P, G, n]
                    csv = cs3[:, :, :n]
                    rb = rg[:].unsqueeze(2).broadcast_to([P, G, n])
                    nc.vector.tensor_mul(csv, col, rb)
                    tt = tpool.tile([P, G * n * n], fp32, name=f"t_{g}_{k}",
                                    tag="t")
                    tv = tt.rearrange("p (g i j) -> p g i j", i=n, j=n)
                    csb = csv.unsqueeze(3).broadcast_to([P, G, n, n])
                    rowb = mv[:, :, k:k + 1, k + 1:].broadcast_to([P, G, n, n])
                    nc.vector.tensor_mul(tv, csb, rowb)
                    sub = mv[:, :, k + 1:, k + 1:]
                    nc.vector.tensor_sub(sub, sub, tv)
                d2 = d2pool.tile([P, G * DIM], fp32, name=f"d2_{g}", tag="d2")
                nc.scalar.activation(d2[:], dview, AF.Square)
                lnd = d2pool.tile([P, G * DIM], fp32, name=f"lnd_{g}", tag="lnd")
                nc.scalar.activation(lnd[:], d2[:], AF.Ln)
                pview = partials[:, g * G:(g + 1) * G].unsqueeze(2)
                nc.vector.tensor_reduce(
                    pview, lnd[:].rearrange("p (g d) -> p g d", d=DIM),
                    axis=AX.X, op=OP.add)

            nc.sync.dma_start(out_d[:], partials[:])
    return nc


def _get(ntiles, group=GROUP):
    key = (ntiles, group)
    if key not in _cache:
        _cache[key] = _build(ntiles, group)
    return _cache[key]


def _legalize_bir(bir_json: bytes) -> bytes:
    """Walrus here allows only ONE embedded sem wait per instruction; split
    extra waits into standalone EventSemaphore instructions (same engine,
    executed in stream order just before the owning instruction)."""
    import json as _json
    j = _json.loads(bir_json)
    n_split = 0
    for fn in j.get("functions", []):
        for blk in fn.get("blocks", []):
            out = []
            for inst in blk.get("instructions", []):
                si = inst.get("sync_info") or {}
                waits = si.get("on_wait") or []
                if len(waits) > 1:
                    for wi, w in enumerate(waits[:-1]):
                        out.append({
                            "debug": 0,
                            "engine": inst.get("engine", "Unassigned"),
                            "ins": [], "outs": [],
                            "name": f"{inst.get('name','I')}-w{wi}",
                            "opcode": "EventSemaphore",
                            "sync_info": {"on_wait": [w], "on_update": []},
                        })
                        n_split += 1
                    si = dict(si)
                    si["on_wait"] = [waits[-1]]
                    inst = dict(inst)
                    inst["sync_info"] = si
                out.append(inst)
            blk["instructions"] = out
    if n_split:
        print(f"[legalize] split {n_split} extra sem waits")
    return _json.dumps(j).encode()


_patched = False


def _install_patch():
    global _patched
    if _patched:
        return
    import concourse.bass_utils as bu
    import concourse.bass2jax as b2j
    orig = bu.compile_bir_kernel

    def patched(bir_json, tmpdir, neff_name="file.neff"):
        return orig(_legalize_bir(bir_json), tmpdir, neff_name)

    bu.compile_bir_kernel = patched
    b2j.compile_bir_kernel = patched
    _patched = True


def _run(x, B, C, ntiles=NTILES_FULL, ncores=NCORES, group=GROUP, trace=False):
    from concourse.bass_utils import run_bass_kernel_spmd
    _install_patch()

    x = np.ascontiguousarray(np.asarray(x, dtype=np.int32))
    B = np.asarray(B, dtype=np.float32)
    C = np.asarray(C, dtype=np.float32)
    eye = np.eye(DIM, dtype=np.float32)
    nshard = ntiles * P
    nc = _get(ntiles, group)
    in_maps = []
    for c in range(ncores):
        in_maps.append({
            "x": x[c * nshard:(c + 1) * nshard],
            "B": B, "C": C, "eye": eye,
        })
    res = run_bass_kernel_spmd(nc, in_maps, core_ids=list(range(ncores)),
                               trace=trace)
    return res


def kernel(x, B, C):
    res = _run(x, B, C)
    total = 0.0
    for r in res.results:
        total += r["out"].astype(np.float64).sum()
    return np.float32(0.5 * total / BATCH)



# revision 2
# speedup vs baseline: 1.2958x; 1.2958x over previous
"""Trainium2 Bass kernel for nn_EstimateGrassmann — prefix-tree version.

Math: p_b = det(sigma - diag(z_b)) with z = 1-x; only the DIAGONAL differs
across samples, so samples sharing the first K bits of z share the first K
Gaussian-elimination steps exactly.  We build a table of all 2^15 prefix
Schur complements (trailing 17x17 matrices + running sum of log(pivot^2)),
then each sample gathers its entry and finishes the last 17 pivots.

Tree construction per core (samples are routed to cores by the top 3 bits
of the prefix, so each core only builds its own 4096-node subtree):
  - bootstrap: each of the 128 partitions eliminates steps 0..9 for its own
    10-bit prefix (3 core bits + 7 partition bits), z supplied as input.
  - levels 10..14: partition-local doubling in the free dim; child slot
    2s+z of parent slot s, so no cross-partition traffic.
  - the level-15 table ([P, 32 * 290]) goes to DRAM; 68 indirect DMAs
    gather per-sample rows back into [P, 17*290] group tiles.

Host side: sort samples by prefix, pad each core's shard to 68*128 = 8704
(real max ~8316), mask the pad slots out of the final mean.
"""

import numpy as np

DIM = 32
BATCH = 65536
NCORES = 8
P = 128
KPRE = 15                  # prefix bits shared via the tree
BOOT = 10                  # bootstrap levels (3 core bits + 7 partition bits)
SLOTS = 1 << (KPRE - BOOT) # 32 table slots per partition
D2 = DIM - KPRE            # 17: trailing matrix size for the sample phase
ROW = D2 * D2 + 1          # 290 floats per table row (matrix + log-partial)
NNODES = P * SLOTS         # 4096 table rows per core
NTILES = 68                # sample capacity per core = 68*128 = 8704
CAP = NTILES * P
GT = 17                    # tiles per elimination group
NGROUPS = NTILES // GT     # 4

ALPHA_C = 1.0 / 4.0
ALPHA_LAM = 1.0 / 5.0
NEWTON_ITERS = 8

_cache = {}


class _PhaseCut(Exception):
    pass


def _build():
    import concourse.bass as bass
    import concourse.mybir as mybir
    from concourse.tile import TileContext

    fp32 = mybir.dt.float32
    i32 = mybir.dt.int32
    AF = mybir.ActivationFunctionType
    OP = mybir.AluOpType
    AX = mybir.AxisListType

    nc = bass.Bass()
    x_d = nc.dram_tensor("x", [CAP, DIM], i32, kind="ExternalInput")
    b_d = nc.dram_tensor("B", [DIM, DIM], fp32, kind="ExternalInput")
    c_d = nc.dram_tensor("C", [DIM, DIM], fp32, kind="ExternalInput")
    eye_d = nc.dram_tensor("eye", [DIM, DIM], fp32, kind="ExternalInput")
    zb_d = nc.dram_tensor("zb", [P, BOOT], fp32, kind="ExternalInput")
    idx_d = nc.dram_tensor("idx", [P, NTILES], i32, kind="ExternalInput")
    out_d = nc.dram_tensor("out", [P, NTILES], fp32, kind="ExternalOutput")

    with TileContext(nc) as tc:
        with tc.tile_pool(name="const", bufs=1) as cpool, \
             tc.tile_pool(name="setup", bufs=1) as spool, \
             tc.tile_pool(name="psum", bufs=2, space="PSUM") as qpool, \
             tc.tile_pool(name="dram", bufs=1, space="DRAM") as dpool, \
             tc.tile_pool(name="tabdram", bufs=1, space="DRAM") as tdpool, \
             tc.tile_pool(name="tree", bufs=1) as tpool, \
             tc.tile_pool(name="big", bufs=1) as gpool, \
             tc.tile_pool(name="r", bufs=4) as rpool, \
             tc.tile_pool(name="d2", bufs=2) as d2pool:

            eye = cpool.tile([DIM, DIM], fp32, name="eye_sb")
            ome = cpool.tile([DIM, DIM], fp32, name="ome_sb")
            eye2 = cpool.tile([DIM, DIM], fp32, name="eye2_sb")
            nc.sync.dma_start(eye[:], eye_d[:])
            nc.vector.tensor_scalar(ome[:], eye[:], -1.0, 1.0,
                                    op0=OP.mult, op1=OP.add)
            nc.vector.tensor_scalar(eye2[:], eye[:], 2.0, None, op0=OP.mult)

            def stabilize(m_sb, nm):
                off = spool.tile([DIM, DIM], fp32, name=f"off_{nm}")
                rl = spool.tile([DIM, DIM], fp32, name=f"rl_{nm}")
                ab = spool.tile([DIM, DIM], fp32, name=f"ab_{nm}")
                ab2 = spool.tile([DIM, DIM], fp32, name=f"ab2_{nm}")
                rs = spool.tile([DIM, 1], fp32, name=f"rs_{nm}")
                rs2 = spool.tile([DIM, 1], fp32, name=f"rs2_{nm}")
                st = spool.tile([DIM, DIM], fp32, name=f"st_{nm}")
                nc.vector.tensor_mul(off[:], m_sb[:], ome[:])
                nc.scalar.activation(rl[:], m_sb[:], AF.Relu)
                nc.scalar.activation(ab[:], off[:], AF.Abs)
                nc.vector.tensor_mul(ab2[:], rl[:], eye[:])
                nc.vector.tensor_reduce(rs[:], ab[:], axis=AX.X, op=OP.add)
                nc.vector.tensor_reduce(rs2[:], ab2[:], axis=AX.X, op=OP.add)
                nc.vector.tensor_add(rs[:], rs[:], rs2[:])
                nc.vector.scalar_tensor_tensor(
                    st[:], eye[:], rs[:, 0:1], off[:], op0=OP.mult, op1=OP.add)
                return st

            def transpose32(src, nm):
                ps = qpool.tile([DIM, DIM], fp32, name=f"pt_{nm}", tag="ps")
                dst = spool.tile([DIM, DIM], fp32, name=f"tr_{nm}")
                nc.tensor.transpose(ps[:], src[:], eye[:])
                nc.scalar.copy(dst[:], ps[:])
                return dst

            def inv32(a_sb, alpha, nm):
                at = transpose32(a_sb, nm)
                xx = spool.tile([DIM, DIM], fp32, name=f"x_{nm}")
                xt = spool.tile([DIM, DIM], fp32, name=f"xt_{nm}")
                nc.vector.tensor_scalar(xx[:], eye[:], alpha, None, op0=OP.mult)
                nc.vector.tensor_scalar(xt[:], eye[:], alpha, None, op0=OP.mult)
                for it in range(NEWTON_ITERS):
                    t1 = qpool.tile([DIM, DIM], fp32, name=f"nt_{nm}_{it}", tag="ps")
                    nc.tensor.matmul(t1[:], at[:], xx[:], start=True, stop=True)
                    w = spool.tile([DIM, DIM], fp32, name=f"w_{nm}_{it}", tag="w")
                    nc.vector.scalar_tensor_tensor(
                        w[:], t1[:], -1.0, eye2[:], op0=OP.mult, op1=OP.add)
                    x2 = qpool.tile([DIM, DIM], fp32, name=f"x2_{nm}_{it}", tag="ps2")
                    xt2 = qpool.tile([DIM, DIM], fp32, name=f"xt2_{nm}_{it}", tag="ps3")
                    nc.tensor.matmul(x2[:], xt[:], w[:], start=True, stop=True)
                    nc.tensor.matmul(xt2[:], w[:], xt[:], start=True, stop=True)
                    nc.scalar.copy(xx[:], x2[:])
                    nc.scalar.copy(xt[:], xt2[:])
                return xx, xt

            # ---- phase A: sigma ----
            b_sb = spool.tile([DIM, DIM], fp32, name="b_sb")
            c_sb = spool.tile([DIM, DIM], fp32, name="c_sb")
            nc.sync.dma_start(b_sb[:], b_d[:])
            nc.sync.dma_start(c_sb[:], c_d[:])
            bs = stabilize(b_sb, "b")
            cs_ = stabilize(c_sb, "c")
            invc, _ = inv32(cs_, ALPHA_C, "c")
            bt = transpose32(bs, "bt")
            lamp = qpool.tile([DIM, DIM], fp32, name="lamp", tag="ps")
            nc.tensor.matmul(lamp[:], bt[:], invc[:], start=True, stop=True)
            lam = spool.tile([DIM, DIM], fp32, name="lam")
            nc.vector.tensor_add(lam[:], lamp[:], eye[:])
            sigma, _ = inv32(lam, ALPHA_LAM, "s")

            sig_dram = dpool.tile([DIM, DIM], fp32, name="sig_dram")
            nc.sync.dma_start(sig_dram[:], sigma[:])
            sig_rep = tpool.tile([P, DIM * DIM], fp32, name="sig_rep")
            src = sig_dram[:].rearrange("a b -> (a b)").unsqueeze(0) \
                             .broadcast_to([P, DIM * DIM])
            nc.sync.dma_start(sig_rep[:], src)

            _done = [False]
            # ---- phase B: bootstrap (steps 0..9, one prefix path per partition)
            zb = spool.tile([P, BOOT], fp32, name="zb_sb")
            nc.sync.dma_start(zb[:], zb_d[:])
            T = tpool.tile([P, DIM * DIM], fp32, name="T_boot")
            nc.vector.tensor_copy(T[:], sig_rep[:])
            Tm = T.rearrange("p (i j) -> p i j", j=DIM)
            Pv = spool.tile([P, BOOT], fp32, name="Pv_boot")
            for k in range(BOOT):
                n = DIM - 1 - k
                nc.vector.tensor_sub(Pv[:, k:k + 1],
                                     T[:, 33 * k:33 * k + 1], zb[:, k:k + 1])
                rv = rpool.tile([P, 1], fp32, name=f"rvb_{k}", tag="rv")
                nc.vector.reciprocal(rv[:], Pv[:, k:k + 1])
                csb_ = rpool.tile([P, n], fp32, name=f"csb_{k}", tag="cs")
                nc.vector.tensor_scalar(csb_[:], Tm[:, k + 1:, k], rv[:, 0:1],
                                        None, op0=OP.mult)
                tvb = d2pool.tile([P, GT * (D2 - 1) * (D2 - 1)], fp32,
                                  name=f"tvb_{k}", tag="tv")[:, :n * n]
                tvv = tvb.rearrange("p (i j) -> p i j", j=n)
                nc.vector.tensor_mul(
                    tvv,
                    csb_[:].unsqueeze(2).broadcast_to([P, n, n]),
                    Tm[:, k:k + 1, k + 1:].broadcast_to([P, n, n]))
                nc.vector.tensor_sub(Tm[:, k + 1:, k + 1:],
                                     Tm[:, k + 1:, k + 1:], tvv)
            d2b = spool.tile([P, BOOT], fp32, name="d2_boot")
            nc.scalar.activation(d2b[:], Pv[:], AF.Square)
            lnb = spool.tile([P, BOOT], fp32, name="ln_boot")
            nc.scalar.activation(lnb[:], d2b[:], AF.Ln)
            bp = spool.tile([P, 1], fp32, name="bp_boot")
            nc.vector.tensor_reduce(bp[:], lnb[:], axis=AX.X, op=OP.add)

            import os as _os
            _phase = _os.environ.get('V2_PHASE', 'full')
            if _phase == 'AB':
                outx = spool.tile([P, NTILES], fp32, name="outx_sb")
                nc.vector.memset(outx[:], 0.0)
                nc.vector.tensor_add(outx[:, 0:1], outx[:, 0:1], bp[:])
                nc.sync.dma_start(out_d[:], outx[:])
                _done[0] = True

            # ---- phase C: doubling levels 10..14 ----
            # tabs[k] holds level-(k+1) children after processing level k
            _crange = [] if _done[0] else range(BOOT, KPRE)
            n10 = DIM - BOOT                      # 22
            # ping-pong buffers for the level tables (X: levels 10/12/14,
            # Y: levels 11/13/15); sized for their largest level
            tabX = tpool.tile([P, 16 * 18 * 18], fp32, name="tabX")
            tabY = tpool.tile([P, SLOTS * ROW], fp32, name="tabY")
            if not _done[0]:
                nc.vector.tensor_copy(
                    tabX[:, :n10 * n10].rearrange("p (i j) -> p i j", j=n10),
                    Tm[:, BOOT:, BOOT:])
                par = spool.tile([P, 1], fp32, name="par10")
                nc.vector.tensor_copy(par[:], bp[:])

            for k in _crange:
                S = 1 << (k - BOOT)
                n = DIM - k
                n2 = n - 1
                last = (k == KPRE - 1)
                stride = ROW if last else n2 * n2
                src_buf = tabX if (k - BOOT) % 2 == 0 else tabY
                dst_buf = tabY if (k - BOOT) % 2 == 0 else tabX
                tab = src_buf[:, :S * n * n]
                tabB = dst_buf[:, :2 * S * stride]
                TmA = tab.rearrange("p (s i j) -> p s i j", i=n, j=n)
                TmB = tabB.rearrange("p (s f) -> p s f", f=stride)
                parB = spool.tile([P, 2 * S], fp32, name=f"par{k + 1}") \
                    if not last else None
                PvL = spool.tile([P, 2 * S], fp32, name=f"PvL{k}")
                # pivots for both children
                nc.vector.tensor_copy(PvL[:, 0::2], TmA[:, :, 0, 0])
                nc.vector.tensor_scalar(PvL[:, 1::2], TmA[:, :, 0, 0], -1.0,
                                        None, op0=OP.add)
                rv = rpool.tile([P, 2 * S], fp32, name=f"rvl_{k}", tag="rv")
                nc.vector.reciprocal(rv[:], PvL[:])
                d2l = d2pool.tile([P, 2 * S], fp32, name=f"d2l_{k}", tag="d2")
                nc.scalar.activation(d2l[:], PvL[:], AF.Square)
                lnl = d2pool.tile([P, 2 * S], fp32, name=f"lnl_{k}", tag="ln")
                nc.scalar.activation(lnl[:], d2l[:], AF.Ln)
                for z in (0, 1):
                    # child partial = parent partial + ln(pivot^2)
                    pdst = TmB[:, z::2, n2 * n2] if last else parB[:, z::2]
                    nc.vector.tensor_add(pdst, par[:], lnl[:, z::2])
                    csl = rpool.tile([P, S * n2], fp32, name=f"csl_{k}_{z}",
                                     tag="cs")
                    cs3 = csl.rearrange("p (s i) -> p s i", i=n2)
                    nc.vector.tensor_mul(
                        cs3, TmA[:, :, 1:, 0],
                        rv[:, z::2].unsqueeze(2).broadcast_to([P, S, n2]))
                    tvl = d2pool.tile([P, S * n2 * n2], fp32,
                                      name=f"tvl_{k}_{z}", tag="tv")
                    tv4 = tvl.rearrange("p (s i j) -> p s i j", i=n2, j=n2)
                    nc.vector.tensor_mul(
                        tv4,
                        cs3[:, :, :].unsqueeze(3).broadcast_to([P, S, n2, n2]),
                        TmA[:, :, 0:1, 1:].broadcast_to([P, S, n2, n2]))
                    child = TmB[:, z::2, 0:n2 * n2] \
                        .rearrange("p s (i j) -> p s i j", j=n2)
                    nc.vector.tensor_sub(child, TmA[:, :, 1:, 1:], tv4)
                par = parB

            # ---- phase D: table to DRAM ----
            table = tdpool.tile([NNODES, ROW], fp32, name="table_dram")
            if not _done[0]:
                nc.sync.dma_start(
                    table[:].rearrange("(p s) f -> p s f", p=P),
                    tabY[:, :SLOTS * ROW].rearrange("p (s f) -> p s f", f=ROW))

            if _phase == 'CD' and not _done[0]:
                outx = spool.tile([P, NTILES], fp32, name="outx_sb")
                nc.vector.memset(outx[:], 0.0)
                nc.sync.dma_start(out_d[:], outx[:])
                _done[0] = True

            # ---- phase E: gather + per-sample elimination ----
            _erange = [] if _done[0] else range(NGROUPS)
            idx = spool.tile([P, NTILES], i32, name="idx_sb")
            nc.sync.dma_start(idx[:], idx_d[:])
            xi = gpool.tile([P, NTILES * D2], i32, name="xi")
            xv = x_d[:].rearrange("(t p) d -> p t d", p=P)[:, :, KPRE:DIM]
            nc.sync.dma_start(
                xi[:].rearrange("p (t d) -> p t d", d=D2), xv)
            zf = gpool.tile([P, NTILES * D2], fp32, name="zf")
            nc.vector.tensor_scalar(zf[:], xi[:], -1.0, 1.0,
                                    op0=OP.mult, op1=OP.add)
            zf3 = zf[:].rearrange("p (t d) -> p t d", d=D2)

            out = spool.tile([P, NTILES], fp32, name="out_sb")

            mgs = []
            for g in _erange:
                mg = gpool.tile([P, GT * ROW], fp32, name=f"m_{g}")
                mgs.append(mg)
                import os
                if os.environ.get('V2_NO_INDIRECT'):
                    nc.sync.dma_start(
                        mg[:].rearrange('p (t f) -> p t f', f=ROW),
                        table[:].rearrange('(p s) f -> p s f', p=P)[:, 0:GT, :])
                else:
                    for t in range(GT):
                        nc.gpsimd.indirect_dma_start(
                            out=mg[:, t * ROW:(t + 1) * ROW],
                            out_offset=None,
                            in_=table[:],
                            in_offset=bass.IndirectOffsetOnAxis(
                                ap=idx[:, g * GT + t:g * GT + t + 1], axis=0),
                        )

            for g in _erange:
                mg = mgs[g]
                m3 = mg.rearrange("p (t f) -> p t f", f=ROW)
                mv = m3[:, :, 0:D2 * D2].rearrange("p t (i j) -> p t i j", j=D2)
                dview = m3[:, :, 0:D2 * D2:D2 + 1]          # [P, GT, 17]
                nc.vector.tensor_sub(dview, dview,
                                     zf3[:, g * GT:(g + 1) * GT, :])
                rg = rpool.tile([P, GT], fp32, name=f"rg_{g}", tag="rg")
                csg = rpool.tile([P, GT * (D2 - 1)], fp32, name=f"cse_{g}",
                                 tag="cse")
                for j in range(D2 - 1):
                    n = D2 - 1 - j
                    nc.vector.reciprocal(rg[:], mv[:, :, j, j])
                    csv = csg.rearrange("p (t i) -> p t i", i=D2 - 1)[:, :, :n]
                    nc.vector.tensor_mul(
                        csv, mv[:, :, j + 1:, j],
                        rg[:].unsqueeze(2).broadcast_to([P, GT, n]))
                    tt = d2pool.tile([P, GT * n * n], fp32, name=f"te_{g}_{j}",
                                     tag="tv")
                    tv4 = tt.rearrange("p (t i j) -> p t i j", i=n, j=n)
                    nc.vector.tensor_mul(
                        tv4,
                        csv.unsqueeze(3).broadcast_to([P, GT, n, n]),
                        mv[:, :, j:j + 1, j + 1:].broadcast_to([P, GT, n, n]))
                    nc.vector.tensor_sub(mv[:, :, j + 1:, j + 1:],
                                         mv[:, :, j + 1:, j + 1:], tv4)
                d2e = d2pool.tile([P, GT * D2], fp32, name=f"d2e_{g}", tag="d2e")
                nc.scalar.activation(d2e[:], dview, AF.Square)
                lne = d2pool.tile([P, GT * D2], fp32, name=f"lne_{g}", tag="lne")
                nc.scalar.activation(lne[:], d2e[:], AF.Ln)
                red = rpool.tile([P, GT], fp32, name=f"red_{g}", tag="red")
                nc.vector.tensor_reduce(
                    red[:].unsqueeze(2),
                    lne[:].rearrange("p (t d) -> p t d", d=D2),
                    axis=AX.X, op=OP.add)
                nc.vector.tensor_add(out[:, g * GT:(g + 1) * GT], red[:],
                                     m3[:, :, D2 * D2])

            if not _done[0]:
                nc.sync.dma_start(out_d[:], out[:])

    import concourse.bass as bass_mod  # noqa: F401
    return nc


def _get():
    if "nc" not in _cache:
        _cache["nc"] = _build()
    return _cache["nc"]


def _legalize_bir(bir_json: bytes) -> bytes:
    """Walrus allows only ONE embedded sem wait per instruction; split extra
    waits into standalone EventSemaphore instructions."""
    import json as _json
    j = _json.loads(bir_json)
    n_split = 0
    for fn in j.get("functions", []):
        for blk in fn.get("blocks", []):
            out = []
            for inst in blk.get("instructions", []):
                si = inst.get("sync_info") or {}
                waits = si.get("on_wait") or []
                if len(waits) > 1:
                    for wi, w in enumerate(waits[:-1]):
                        out.append({
                            "debug": 0,
                            "engine": inst.get("engine", "Unassigned"),
                            "ins": [], "outs": [],
                            "name": f"{inst.get('name','I')}-w{wi}",
                            "opcode": "EventSemaphore",
                            "sync_info": {"on_wait": [w], "on_update": []},
                        })
                        n_split += 1
                    si = dict(si)
                    si["on_wait"] = [waits[-1]]
                    inst = dict(inst)
                    inst["sync_info"] = si
                out.append(inst)
            blk["instructions"] = out
    if n_split:
        print(f"[legalize] split {n_split} extra sem waits")
    return _json.dumps(j).encode()


_patched = False


def _install_patch():
    global _patched
    if _patched:
        return
    import concourse.bass_utils as bu
    import concourse.bass2jax as b2j
    orig = bu.compile_bir_kernel

    def patched(bir_json, tmpdir, neff_name="file.neff"):
        return orig(_legalize_bir(bir_json), tmpdir, neff_name)

    bu.compile_bir_kernel = patched
    b2j.compile_bir_kernel = patched
    _patched = True


def _preprocess(x):
    """Sort samples by 15-bit prefix, route to cores by top 3 bits, pad."""
    x = np.ascontiguousarray(np.asarray(x, dtype=np.int32))
    z = (1 - x).astype(np.int64)
    prefix = np.zeros(len(x), dtype=np.int64)
    for k in range(KPRE):
        prefix = (prefix << 1) | z[:, k]
    core = (prefix >> (KPRE - 3)).astype(np.int64)
    row = (prefix & (NNODES - 1)).astype(np.int32)

    zb = np.zeros((NCORES, P, BOOT), dtype=np.float32)
    for c in range(NCORES):
        node = c * P + np.arange(P)
        for k in range(BOOT):
            zb[c, :, k] = (node >> (BOOT - 1 - k)) & 1

    xs, idxs, counts = [], [], []
    for c in range(NCORES):
        sel = np.nonzero(core == c)[0]
        ncs = len(sel)
        assert ncs <= CAP, f"core {c} overflow: {ncs} > {CAP}"
        xc = np.empty((CAP, DIM), dtype=np.int32)
        rc = np.empty(CAP, dtype=np.int32)
        xc[:ncs] = x[sel]
        rc[:ncs] = row[sel]
        if ncs < CAP:
            xc[ncs:] = xc[0]
            rc[ncs:] = rc[0]
        # out[p, t] corresponds to shard row t*P + p
        idx_pt = rc.reshape(NTILES, P).T.copy()
        xs.append(xc)
        idxs.append(idx_pt)
        counts.append(ncs)
    return xs, idxs, zb, counts


def _run(x, B, C, ncores=NCORES, trace=False):
    from concourse.bass_utils import run_bass_kernel_spmd
    _install_patch()

    B = np.asarray(B, dtype=np.float32)
    C = np.asarray(C, dtype=np.float32)
    eye = np.eye(DIM, dtype=np.float32)
    xs, idxs, zb, counts = _preprocess(x)
    nc = _get()
    in_maps = []
    for c in range(ncores):
        in_maps.append({
            "x": xs[c], "B": B, "C": C, "eye": eye,
            "zb": zb[c], "idx": idxs[c],
        })
    res = run_bass_kernel_spmd(nc, in_maps, core_ids=list(range(ncores)),
                               trace=trace)
    return res, counts


def _reduce(res_results, counts):
    total = 0.0
    for c, r in enumerate(res_results):
        o = r["out"]                       # [P, NTILES]
        ncs = counts[c]
        vals = o.T.reshape(-1)             # slot s = t*P+p -> o[p, t]
        total += vals[:ncs].astype(np.float64).sum()
    return np.float32(0.5 * total / BATCH)


def kernel(x, B, C):
    res, counts = _run(x, B, C)
    return _reduce(res.results, counts)


# revision 3
# speedup vs baseline: 3.1992x; 2.4689x over previous
"""Trainium2 Bass kernel for nn_EstimateGrassmann — prefix-tree version.

Math: p_b = det(sigma - diag(z_b)) with z = 1-x; only the DIAGONAL differs
across samples, so samples sharing the first K bits of z share the first K
Gaussian-elimination steps exactly.  We build a per-core table of all 4096
local prefix Schur complements (trailing 17x17 matrices + running sum of
log(pivot^2)), then each sample gathers its entry via indirect DMA and
finishes the last 17 pivots.

Tree construction per core (samples are routed to cores by the top 3 bits
of the 15-bit prefix, so each core only builds its own 4096-node subtree):
  - bootstrap: each of the 128 partitions eliminates steps 0..9 for its own
    10-bit prefix (3 core bits + 7 partition bits), z supplied as input.
  - levels 10..14: partition-local doubling in the free dim; child slot
    2s+z of parent slot s, so no cross-partition traffic.
  - the level-15 table ([P, 32*290]) goes to DRAM; 68 per-tile indirect
    DMAs gather per-sample rows back into [P, 17*290] group tiles.

Host side: sort samples by prefix, pad each core's shard to 68*128 = 8704
(real max ~8316 for Binomial(65536, 1/8)), mask pad slots out of the mean.
All per-core inputs are packed into ONE int32 [9112, 32] blob — per-call
overhead on the axon/PJRT path scales with the number of input buffers
(~1 ms each), dwarfing the ~0.5 ms device time otherwise.
"""

import numpy as np

DIM = 32
BATCH = 65536
NCORES = 8
P = 128
KPRE = 15                  # prefix bits shared via the tree
BOOT = 10                  # bootstrap levels (3 core bits + 7 partition bits)
SLOTS = 1 << (KPRE - BOOT) # 32 table slots per partition
D2 = DIM - KPRE            # 17: trailing matrix size for the sample phase
ROW = D2 * D2 + 1          # 290 floats per table row (matrix + log-partial)
NNODES = P * SLOTS         # 4096 table rows per core
NTILES = 68                # sample capacity per core = 68*128 = 8704
CAP = NTILES * P
GT = 17                    # tiles per elimination group
NGROUPS = NTILES // GT     # 4

# packed input blob layout (rows of 32 int32 words)
ROW_B = CAP                # B  [32, 32] f32 (bitcast)
ROW_C = CAP + 32           # C  [32, 32] f32
ROW_EYE = CAP + 64         # eye [32, 32] f32
ROW_ZB = CAP + 96          # zb [128, 10] f32 -> 40 rows
ROW_IDX = CAP + 136        # idx [128, 68] i32 -> 272 rows
BLOB_ROWS = CAP + 136 + (P * NTILES) // DIM   # 9112

ALPHA_C = 1.0 / 4.0
ALPHA_LAM = 1.0 / 5.0
NEWTON_ITERS = 8

_cache = {}


def _build():
    import concourse.bass as bass
    import concourse.mybir as mybir
    from concourse.tile import TileContext

    fp32 = mybir.dt.float32
    i32 = mybir.dt.int32
    AF = mybir.ActivationFunctionType
    OP = mybir.AluOpType
    AX = mybir.AxisListType

    nc = bass.Bass()
    blob_d = nc.dram_tensor("blob", [BLOB_ROWS, DIM], i32, kind="ExternalInput")
    out_d = nc.dram_tensor("out", [P, NTILES], fp32, kind="ExternalOutput")

    b_src = blob_d[ROW_B:ROW_B + 32, :].bitcast(fp32)
    c_src = blob_d[ROW_C:ROW_C + 32, :].bitcast(fp32)
    eye_src = blob_d[ROW_EYE:ROW_EYE + 32, :].bitcast(fp32)
    zb_src = blob_d[ROW_ZB:ROW_ZB + 40, :].bitcast(fp32) \
        .rearrange("a b -> (a b)").rearrange("(p k) -> p k", k=BOOT)
    idx_src = blob_d[ROW_IDX:ROW_IDX + 272, :] \
        .rearrange("a b -> (a b)").rearrange("(p k) -> p k", k=NTILES)

    with TileContext(nc) as tc:
        with tc.tile_pool(name="const", bufs=1) as cpool, \
             tc.tile_pool(name="setup", bufs=1) as spool, \
             tc.tile_pool(name="psum", bufs=2, space="PSUM") as qpool, \
             tc.tile_pool(name="dram", bufs=1, space="DRAM") as dpool, \
             tc.tile_pool(name="tabdram", bufs=1, space="DRAM") as tdpool, \
             tc.tile_pool(name="tree", bufs=1) as tpool, \
             tc.tile_pool(name="big", bufs=1) as gpool, \
             tc.tile_pool(name="r", bufs=4) as rpool, \
             tc.tile_pool(name="d2", bufs=2) as d2pool:

            eye = cpool.tile([DIM, DIM], fp32, name="eye_sb")
            ome = cpool.tile([DIM, DIM], fp32, name="ome_sb")
            eye2 = cpool.tile([DIM, DIM], fp32, name="eye2_sb")
            nc.sync.dma_start(eye[:], eye_src)
            nc.vector.tensor_scalar(ome[:], eye[:], -1.0, 1.0,
                                    op0=OP.mult, op1=OP.add)
            nc.vector.tensor_scalar(eye2[:], eye[:], 2.0, None, op0=OP.mult)

            def stabilize(m_sb, nm):
                off = spool.tile([DIM, DIM], fp32, name=f"off_{nm}")
                rl = spool.tile([DIM, DIM], fp32, name=f"rl_{nm}")
                ab = spool.tile([DIM, DIM], fp32, name=f"ab_{nm}")
                ab2 = spool.tile([DIM, DIM], fp32, name=f"ab2_{nm}")
                rs = spool.tile([DIM, 1], fp32, name=f"rs_{nm}")
                rs2 = spool.tile([DIM, 1], fp32, name=f"rs2_{nm}")
                st = spool.tile([DIM, DIM], fp32, name=f"st_{nm}")
                nc.vector.tensor_mul(off[:], m_sb[:], ome[:])
                nc.scalar.activation(rl[:], m_sb[:], AF.Relu)
                nc.scalar.activation(ab[:], off[:], AF.Abs)
                nc.vector.tensor_mul(ab2[:], rl[:], eye[:])
                nc.vector.tensor_reduce(rs[:], ab[:], axis=AX.X, op=OP.add)
                nc.vector.tensor_reduce(rs2[:], ab2[:], axis=AX.X, op=OP.add)
                nc.vector.tensor_add(rs[:], rs[:], rs2[:])
                nc.vector.scalar_tensor_tensor(
                    st[:], eye[:], rs[:, 0:1], off[:], op0=OP.mult, op1=OP.add)
                return st

            def transpose32(src_sb, nm):
                ps = qpool.tile([DIM, DIM], fp32, name=f"pt_{nm}", tag="ps")
                dst = spool.tile([DIM, DIM], fp32, name=f"tr_{nm}")
                nc.tensor.transpose(ps[:], src_sb[:], eye[:])
                nc.scalar.copy(dst[:], ps[:])
                return dst

            def inv32(a_sb, alpha, nm):
                at = transpose32(a_sb, nm)
                xx = spool.tile([DIM, DIM], fp32, name=f"x_{nm}")
                xt = spool.tile([DIM, DIM], fp32, name=f"xt_{nm}")
                nc.vector.tensor_scalar(xx[:], eye[:], alpha, None, op0=OP.mult)
                nc.vector.tensor_scalar(xt[:], eye[:], alpha, None, op0=OP.mult)
                for it in range(NEWTON_ITERS):
                    t1 = qpool.tile([DIM, DIM], fp32, name=f"nt_{nm}_{it}", tag="ps")
                    nc.tensor.matmul(t1[:], at[:], xx[:], start=True, stop=True)
                    w = spool.tile([DIM, DIM], fp32, name=f"w_{nm}_{it}", tag="w")
                    nc.vector.scalar_tensor_tensor(
                        w[:], t1[:], -1.0, eye2[:], op0=OP.mult, op1=OP.add)
                    x2 = qpool.tile([DIM, DIM], fp32, name=f"x2_{nm}_{it}", tag="ps2")
                    xt2 = qpool.tile([DIM, DIM], fp32, name=f"xt2_{nm}_{it}", tag="ps3")
                    nc.tensor.matmul(x2[:], xt[:], w[:], start=True, stop=True)
                    nc.tensor.matmul(xt2[:], w[:], xt[:], start=True, stop=True)
                    nc.scalar.copy(xx[:], x2[:])
                    nc.scalar.copy(xt[:], xt2[:])
                return xx, xt

            # ---- phase A: sigma = inv(stab(B) @ inv(stab(C)) + I) ----
            b_sb = spool.tile([DIM, DIM], fp32, name="b_sb")
            c_sb = spool.tile([DIM, DIM], fp32, name="c_sb")
            nc.sync.dma_start(b_sb[:], b_src)
            nc.sync.dma_start(c_sb[:], c_src)
            bs = stabilize(b_sb, "b")
            cs_ = stabilize(c_sb, "c")
            invc, _ = inv32(cs_, ALPHA_C, "c")
            bt = transpose32(bs, "bt")
            lamp = qpool.tile([DIM, DIM], fp32, name="lamp", tag="ps")
            nc.tensor.matmul(lamp[:], bt[:], invc[:], start=True, stop=True)
            lam = spool.tile([DIM, DIM], fp32, name="lam")
            nc.vector.tensor_add(lam[:], lamp[:], eye[:])
            sigma, _ = inv32(lam, ALPHA_LAM, "s")

            sig_dram = dpool.tile([DIM, DIM], fp32, name="sig_dram")
            nc.sync.dma_start(sig_dram[:], sigma[:])
            sig_rep = tpool.tile([P, DIM * DIM], fp32, name="sig_rep")
            src = sig_dram[:].rearrange("a b -> (a b)").unsqueeze(0) \
                             .broadcast_to([P, DIM * DIM])
            nc.sync.dma_start(sig_rep[:], src)

            # ---- phase B: bootstrap (steps 0..9, one prefix path/partition)
            zb = spool.tile([P, BOOT], fp32, name="zb_sb")
            nc.sync.dma_start(zb[:], zb_src)
            T = tpool.tile([P, DIM * DIM], fp32, name="T_boot")
            nc.vector.tensor_copy(T[:], sig_rep[:])
            Tm = T.rearrange("p (i j) -> p i j", j=DIM)
            Pv = spool.tile([P, BOOT], fp32, name="Pv_boot")
            for k in range(BOOT):
                n = DIM - 1 - k
                nc.vector.tensor_sub(Pv[:, k:k + 1],
                                     T[:, 33 * k:33 * k + 1], zb[:, k:k + 1])
                rv = rpool.tile([P, 1], fp32, name=f"rvb_{k}", tag="rv")
                nc.vector.reciprocal(rv[:], Pv[:, k:k + 1])
                csb_ = rpool.tile([P, n], fp32, name=f"csb_{k}", tag="cs")
                nc.vector.tensor_scalar(csb_[:], Tm[:, k + 1:, k], rv[:, 0:1],
                                        None, op0=OP.mult)
                tvb = d2pool.tile([P, GT * (D2 - 1) * (D2 - 1)], fp32,
                                  name=f"tvb_{k}", tag="tv")[:, :n * n]
                tvv = tvb.rearrange("p (i j) -> p i j", j=n)
                nc.vector.tensor_mul(
                    tvv,
                    csb_[:].unsqueeze(2).broadcast_to([P, n, n]),
                    Tm[:, k:k + 1, k + 1:].broadcast_to([P, n, n]))
                nc.vector.tensor_sub(Tm[:, k + 1:, k + 1:],
                                     Tm[:, k + 1:, k + 1:], tvv)
            d2b = spool.tile([P, BOOT], fp32, name="d2_boot")
            nc.scalar.activation(d2b[:], Pv[:], AF.Square)
            lnb = spool.tile([P, BOOT], fp32, name="ln_boot")
            nc.scalar.activation(lnb[:], d2b[:], AF.Ln)
            bp = spool.tile([P, 1], fp32, name="bp_boot")
            nc.vector.tensor_reduce(bp[:], lnb[:], axis=AX.X, op=OP.add)

            # ---- phase C: doubling levels 10..14 ----
            n10 = DIM - BOOT                      # 22
            tabX = tpool.tile([P, 16 * 18 * 18], fp32, name="tabX")
            tabY = tpool.tile([P, SLOTS * ROW], fp32, name="tabY")
            nc.vector.tensor_copy(
                tabX[:, :n10 * n10].rearrange("p (i j) -> p i j", j=n10),
                Tm[:, BOOT:, BOOT:])
            par = spool.tile([P, 1], fp32, name="par10")
            nc.vector.tensor_copy(par[:], bp[:])

            for k in range(BOOT, KPRE):
                S = 1 << (k - BOOT)
                n = DIM - k
                n2 = n - 1
                last = (k == KPRE - 1)
                stride = ROW if last else n2 * n2
                src_buf = tabX if (k - BOOT) % 2 == 0 else tabY
                dst_buf = tabY if (k - BOOT) % 2 == 0 else tabX
                tab = src_buf[:, :S * n * n]
                tabB = dst_buf[:, :2 * S * stride]
                TmA = tab.rearrange("p (s i j) -> p s i j", i=n, j=n)
                TmB = tabB.rearrange("p (s f) -> p s f", f=stride)
                parB = spool.tile([P, 2 * S], fp32, name=f"par{k + 1}") \
                    if not last else None
                PvL = spool.tile([P, 2 * S], fp32, name=f"PvL{k}")
                nc.vector.tensor_copy(PvL[:, 0::2], TmA[:, :, 0, 0])
                nc.vector.tensor_scalar(PvL[:, 1::2], TmA[:, :, 0, 0], -1.0,
                                        None, op0=OP.add)
                rv = rpool.tile([P, 2 * S], fp32, name=f"rvl_{k}", tag="rv")
                nc.vector.reciprocal(rv[:], PvL[:])
                d2l = d2pool.tile([P, 2 * S], fp32, name=f"d2l_{k}", tag="d2")
                nc.scalar.activation(d2l[:], PvL[:], AF.Square)
                lnl = d2pool.tile([P, 2 * S], fp32, name=f"lnl_{k}", tag="ln")
                nc.scalar.activation(lnl[:], d2l[:], AF.Ln)
                for z in (0, 1):
                    pdst = TmB[:, z::2, n2 * n2] if last else parB[:, z::2]
                    nc.vector.tensor_add(pdst, par[:], lnl[:, z::2])
                    csl = rpool.tile([P, S * n2], fp32, name=f"csl_{k}_{z}",
                                     tag="cs")
                    cs3 = csl.rearrange("p (s i) -> p s i", i=n2)
                    nc.vector.tensor_mul(
                        cs3, TmA[:, :, 1:, 0],
                        rv[:, z::2].unsqueeze(2).broadcast_to([P, S, n2]))
                    tvl = d2pool.tile([P, S * n2 * n2], fp32,
                                      name=f"tvl_{k}_{z}", tag="tv")
                    tv4 = tvl.rearrange("p (s i j) -> p s i j", i=n2, j=n2)
                    nc.vector.tensor_mul(
                        tv4,
                        cs3[:, :, :].unsqueeze(3).broadcast_to([P, S, n2, n2]),
                        TmA[:, :, 0:1, 1:].broadcast_to([P, S, n2, n2]))
                    child = TmB[:, z::2, 0:n2 * n2] \
                        .rearrange("p s (i j) -> p s i j", j=n2)
                    nc.vector.tensor_sub(child, TmA[:, :, 1:, 1:], tv4)
                par = parB

            # ---- phase D: table to DRAM ----
            table = tdpool.tile([NNODES, ROW], fp32, name="table_dram")
            nc.sync.dma_start(
                table[:].rearrange("(p s) f -> p s f", p=P),
                tabY[:, :SLOTS * ROW].rearrange("p (s f) -> p s f", f=ROW))

            # ---- phase E: gather + per-sample elimination ----
            idx = spool.tile([P, NTILES], i32, name="idx_sb")
            nc.sync.dma_start(idx[:], idx_src)
            xi = gpool.tile([P, NTILES * D2], i32, name="xi")
            xv = blob_d[0:CAP, :].rearrange("(t p) d -> p t d", p=P)[:, :, KPRE:DIM]
            nc.sync.dma_start(
                xi[:].rearrange("p (t d) -> p t d", d=D2), xv)
            zf = gpool.tile([P, NTILES * D2], fp32, name="zf")
            nc.vector.tensor_scalar(zf[:], xi[:], -1.0, 1.0,
                                    op0=OP.mult, op1=OP.add)
            zf3 = zf[:].rearrange("p (t d) -> p t d", d=D2)

            out = spool.tile([P, NTILES], fp32, name="out_sb")

            mgs = []
            for g in range(NGROUPS):
                mg = gpool.tile([P, GT * ROW], fp32, name=f"m_{g}")
                mgs.append(mg)
                for t in range(GT):
                    nc.gpsimd.indirect_dma_start(
                        out=mg[:, t * ROW:(t + 1) * ROW],
                        out_offset=None,
                        in_=table[:],
                        in_offset=bass.IndirectOffsetOnAxis(
                            ap=idx[:, g * GT + t:g * GT + t + 1], axis=0),
                    )

            for g in range(NGROUPS):
                mg = mgs[g]
                m3 = mg.rearrange("p (t f) -> p t f", f=ROW)
                mv = m3[:, :, 0:D2 * D2].rearrange("p t (i j) -> p t i j", j=D2)
                dview = m3[:, :, 0:D2 * D2:D2 + 1]          # [P, GT, 17]
                nc.vector.tensor_sub(dview, dview,
                                     zf3[:, g * GT:(g + 1) * GT, :])
                rg = rpool.tile([P, GT], fp32, name=f"rg_{g}", tag="rg")
                csg = rpool.tile([P, GT * (D2 - 1)], fp32, name=f"cse_{g}",
                                 tag="cse")
                for j in range(D2 - 1):
                    n = D2 - 1 - j
                    nc.vector.reciprocal(rg[:], mv[:, :, j, j])
                    csv = csg.rearrange("p (t i) -> p t i", i=D2 - 1)[:, :, :n]
                    nc.vector.tensor_mul(
                        csv, mv[:, :, j + 1:, j],
                        rg[:].unsqueeze(2).broadcast_to([P, GT, n]))
                    tt = d2pool.tile([P, GT * n * n], fp32, name=f"te_{g}_{j}",
                                     tag="tv")
                    tv4 = tt.rearrange("p (t i j) -> p t i j", i=n, j=n)
                    nc.vector.tensor_mul(
                        tv4,
                        csv.unsqueeze(3).broadcast_to([P, GT, n, n]),
                        mv[:, :, j:j + 1, j + 1:].broadcast_to([P, GT, n, n]))
                    nc.vector.tensor_sub(mv[:, :, j + 1:, j + 1:],
                                         mv[:, :, j + 1:, j + 1:], tv4)
                d2e = d2pool.tile([P, GT * D2], fp32, name=f"d2e_{g}", tag="d2")
                nc.scalar.activation(d2e[:], dview, AF.Square)
                lne = d2pool.tile([P, GT * D2], fp32, name=f"lne_{g}", tag="ln")
                nc.scalar.activation(lne[:], d2e[:], AF.Ln)
                red = rpool.tile([P, GT], fp32, name=f"red_{g}", tag="red")
                nc.vector.tensor_reduce(
                    red[:].unsqueeze(2),
                    lne[:].rearrange("p (t d) -> p t d", d=D2),
                    axis=AX.X, op=OP.add)
                nc.vector.tensor_add(out[:, g * GT:(g + 1) * GT], red[:],
                                     m3[:, :, D2 * D2])

            nc.sync.dma_start(out_d[:], out[:])

    return nc


def _get():
    if "nc" not in _cache:
        _cache["nc"] = _build()
    return _cache["nc"]


def _legalize_bir(bir_json: bytes) -> bytes:
    """Walrus allows only ONE embedded sem wait per instruction; split extra
    waits into standalone EventSemaphore instructions."""
    import json as _json
    j = _json.loads(bir_json)
    n_split = 0
    for fn in j.get("functions", []):
        for blk in fn.get("blocks", []):
            out = []
            for inst in blk.get("instructions", []):
                si = inst.get("sync_info") or {}
                waits = si.get("on_wait") or []
                if len(waits) > 1:
                    for wi, w in enumerate(waits[:-1]):
                        out.append({
                            "debug": 0,
                            "engine": inst.get("engine", "Unassigned"),
                            "ins": [], "outs": [],
                            "name": f"{inst.get('name','I')}-w{wi}",
                            "opcode": "EventSemaphore",
                            "sync_info": {"on_wait": [w], "on_update": []},
                        })
                        n_split += 1
                    si = dict(si)
                    si["on_wait"] = [waits[-1]]
                    inst = dict(inst)
                    inst["sync_info"] = si
                out.append(inst)
            blk["instructions"] = out
    if n_split:
        print(f"[legalize] split {n_split} extra sem waits")
    return _json.dumps(j).encode()


_patched = False


def _install_patch():
    global _patched
    if _patched:
        return
    import concourse.bass_utils as bu
    import concourse.bass2jax as b2j
    orig = bu.compile_bir_kernel

    def patched(bir_json, tmpdir, neff_name="file.neff"):
        return orig(_legalize_bir(bir_json), tmpdir, neff_name)

    bu.compile_bir_kernel = patched
    b2j.compile_bir_kernel = patched
    _patched = True


def _preprocess(x, B, C):
    """Sort samples by 15-bit prefix, route to cores by top 3 bits, pad,
    and pack each core's inputs into one int32 blob."""
    x = np.ascontiguousarray(np.asarray(x, dtype=np.int32))
    B = np.asarray(B, dtype=np.float32)
    C = np.asarray(C, dtype=np.float32)
    eye = np.eye(DIM, dtype=np.float32)
    z = (1 - x).astype(np.int64)
    prefix = np.zeros(len(x), dtype=np.int64)
    for k in range(KPRE):
        prefix = (prefix << 1) | z[:, k]
    core = (prefix >> (KPRE - 3)).astype(np.int64)
    row = (prefix & (NNODES - 1)).astype(np.int32)

    blobs, counts = [], []
    for c in range(NCORES):
        sel = np.nonzero(core == c)[0]
        ncs = len(sel)
        assert ncs <= CAP, f"core {c} overflow: {ncs} > {CAP}"
        xc = np.empty((CAP, DIM), dtype=np.int32)
        rc = np.empty(CAP, dtype=np.int32)
        xc[:ncs] = x[sel]
        rc[:ncs] = row[sel]
        if ncs < CAP:
            xc[ncs:] = xc[0]
            rc[ncs:] = rc[0]
        idx_pt = rc.reshape(NTILES, P).T   # out[p, t] <-> shard row t*P+p
        node = c * P + np.arange(P)
        zb = np.empty((P, BOOT), dtype=np.float32)
        for k in range(BOOT):
            zb[:, k] = (node >> (BOOT - 1 - k)) & 1

        blob = np.empty((BLOB_ROWS, DIM), dtype=np.int32)
        blob[0:CAP] = xc
        blob[ROW_B:ROW_B + 32] = B.view(np.int32)
        blob[ROW_C:ROW_C + 32] = C.view(np.int32)
        blob[ROW_EYE:ROW_EYE + 32] = eye.view(np.int32)
        blob[ROW_ZB:ROW_ZB + 40] = zb.reshape(-1).view(np.int32).reshape(40, DIM)
        blob[ROW_IDX:ROW_IDX + 272] = \
            np.ascontiguousarray(idx_pt).reshape(-1).reshape(272, DIM)
        blobs.append(blob)
        counts.append(ncs)
    return blobs, counts


def _run(x, B, C, ncores=NCORES, trace=False):
    from concourse.bass_utils import run_bass_kernel_spmd
    _install_patch()

    blobs, counts = _preprocess(x, B, C)
    nc = _get()
    in_maps = [{"blob": blobs[c]} for c in range(ncores)]
    res = run_bass_kernel_spmd(nc, in_maps, core_ids=list(range(ncores)),
                               trace=trace)
    return res, counts


def _reduce(res_results, counts):
    total = 0.0
    for c, r in enumerate(res_results):
        o = r["out"]                       # [P, NTILES]
        ncs = counts[c]
        vals = o.T.reshape(-1)             # slot s = t*P+p -> o[p, t]
        total += vals[:ncs].astype(np.float64).sum()
    return np.float32(0.5 * total / BATCH)


def kernel(x, B, C):
    res, counts = _run(x, B, C)
    return _reduce(res.results, counts)


# revision 6
# speedup vs baseline: 3.7202x; 1.1628x over previous
"""Trainium2 Bass kernel for nn_EstimateGrassmann — prefix-tree version.

Math: p_b = det(sigma - diag(z_b)) with z = 1-x; only the DIAGONAL differs
across samples, so samples sharing the first K bits of z share the first K
Gaussian-elimination steps exactly.  We build a per-core table of all 4096
local prefix Schur complements (trailing 17x17 matrices + running sum of
log(pivot^2)), then each sample gathers its entry via indirect DMA and
finishes the last 17 pivots.

Tree construction per core (samples are routed to cores by the top 3 bits
of the 15-bit prefix, so each core only builds its own 4096-node subtree):
  - bootstrap: each of the 128 partitions eliminates steps 0..9 for its own
    10-bit prefix (3 core bits + 7 partition bits), z supplied as input.
  - levels 10..14: partition-local doubling in the free dim; child slot
    2s+z of parent slot s, so no cross-partition traffic.
  - the level-15 table ([P, 32*290]) goes to DRAM; 68 per-tile indirect
    DMAs gather per-sample rows back into [P, 17*290] group tiles.

Host side: sort samples by prefix, pad each core's shard to 68*128 = 8704
(real max ~8316 for Binomial(65536, 1/8)), mask pad slots out of the mean.
All per-core inputs are packed into ONE int32 [9112, 32] blob — per-call
overhead on the axon/PJRT path scales with the number of input buffers
(~1 ms each), dwarfing the ~0.5 ms device time otherwise.
"""

import numpy as np

DIM = 32
BATCH = 65536
NCORES = 8
P = 128
KPRE = 15                  # prefix bits shared via the tree
BOOT = 10                  # bootstrap levels (3 core bits + 7 partition bits)
SLOTS = 1 << (KPRE - BOOT) # 32 table slots per partition
D2 = DIM - KPRE            # 17: trailing matrix size for the sample phase
ROW = D2 * D2 + 1          # 290 floats per table row (matrix + log-partial)
NNODES = P * SLOTS         # 4096 table rows per core
NTILES = 68                # sample capacity per core = 68*128 = 8704
CAP = NTILES * P
GT = 17                    # tiles per elimination group
NGROUPS = NTILES // GT     # 4

# packed input blob layout (rows of 32 int32 words)
ZF_ROWS = (P * NTILES * D2) // DIM  # zf [128, 68*17] f32 -> 4624 rows
ROW_B = ZF_ROWS            # B  [32, 32] f32 (bitcast)
ROW_C = ZF_ROWS + 32       # C  [32, 32] f32
ROW_EYE = ZF_ROWS + 64     # eye [32, 32] f32
ROW_ZB = ZF_ROWS + 96      # zb [128, 10] f32 -> 40 rows
ROW_IDX = ZF_ROWS + 136    # idx [128, 68] i32 -> 272 rows
BLOB_ROWS = ZF_ROWS + 136 + (P * NTILES) // DIM   # 5032

ALPHA_C = 1.0 / 4.0
ALPHA_LAM = 1.0 / 5.0
NEWTON_ITERS = 8

_cache = {}


def _build():
    import concourse.bass as bass
    import concourse.mybir as mybir
    from concourse.tile import TileContext

    fp32 = mybir.dt.float32
    i32 = mybir.dt.int32
    AF = mybir.ActivationFunctionType
    OP = mybir.AluOpType
    AX = mybir.AxisListType

    nc = bass.Bass()
    blob_d = nc.dram_tensor("blob", [BLOB_ROWS, DIM], i32, kind="ExternalInput")
    out_d = nc.dram_tensor("out", [P, NTILES], fp32, kind="ExternalOutput")

    zf_src = blob_d[0:ZF_ROWS, :].bitcast(fp32) \
        .rearrange("a b -> (a b)").rearrange("(p f) -> p f", f=NTILES * D2)
    b_src = blob_d[ROW_B:ROW_B + 32, :].bitcast(fp32)
    c_src = blob_d[ROW_C:ROW_C + 32, :].bitcast(fp32)
    eye_src = blob_d[ROW_EYE:ROW_EYE + 32, :].bitcast(fp32)
    zb_src = blob_d[ROW_ZB:ROW_ZB + 40, :].bitcast(fp32) \
        .rearrange("a b -> (a b)").rearrange("(p k) -> p k", k=BOOT)
    idx_src = blob_d[ROW_IDX:ROW_IDX + 272, :] \
        .rearrange("a b -> (a b)").rearrange("(p k) -> p k", k=NTILES)

    with TileContext(nc) as tc:
        with tc.tile_pool(name="const", bufs=1) as cpool, \
             tc.tile_pool(name="setup", bufs=1) as spool, \
             tc.tile_pool(name="psum", bufs=2, space="PSUM") as qpool, \
             tc.tile_pool(name="dram", bufs=1, space="DRAM") as dpool, \
             tc.tile_pool(name="tabdram", bufs=1, space="DRAM") as tdpool, \
             tc.tile_pool(name="tree", bufs=1) as tpool, \
             tc.tile_pool(name="big", bufs=1) as gpool, \
             tc.tile_pool(name="r", bufs=4) as rpool, \
             tc.tile_pool(name="d2", bufs=2) as d2pool:

            eye = cpool.tile([DIM, DIM], fp32, name="eye_sb")
            ome = cpool.tile([DIM, DIM], fp32, name="ome_sb")
            eye2 = cpool.tile([DIM, DIM], fp32, name="eye2_sb")
            nc.sync.dma_start(eye[:], eye_src)
            nc.vector.tensor_scalar(ome[:], eye[:], -1.0, 1.0,
                                    op0=OP.mult, op1=OP.add)
            nc.vector.tensor_scalar(eye2[:], eye[:], 2.0, None, op0=OP.mult)

            def stabilize(m_sb, nm):
                off = spool.tile([DIM, DIM], fp32, name=f"off_{nm}")
                rl = spool.tile([DIM, DIM], fp32, name=f"rl_{nm}")
                ab = spool.tile([DIM, DIM], fp32, name=f"ab_{nm}")
                ab2 = spool.tile([DIM, DIM], fp32, name=f"ab2_{nm}")
                rs = spool.tile([DIM, 1], fp32, name=f"rs_{nm}")
                rs2 = spool.tile([DIM, 1], fp32, name=f"rs2_{nm}")
                st = spool.tile([DIM, DIM], fp32, name=f"st_{nm}")
                nc.vector.tensor_mul(off[:], m_sb[:], ome[:])
                nc.scalar.activation(rl[:], m_sb[:], AF.Relu)
                nc.scalar.activation(ab[:], off[:], AF.Abs)
                nc.vector.tensor_mul(ab2[:], rl[:], eye[:])
                nc.vector.tensor_reduce(rs[:], ab[:], axis=AX.X, op=OP.add)
                nc.vector.tensor_reduce(rs2[:], ab2[:], axis=AX.X, op=OP.add)
                nc.vector.tensor_add(rs[:], rs[:], rs2[:])
                nc.vector.scalar_tensor_tensor(
                    st[:], eye[:], rs[:, 0:1], off[:], op0=OP.mult, op1=OP.add)
                return st

            def transpose32(src_sb, nm):
                ps = qpool.tile([DIM, DIM], fp32, name=f"pt_{nm}", tag="ps")
                dst = spool.tile([DIM, DIM], fp32, name=f"tr_{nm}")
                nc.tensor.transpose(ps[:], src_sb[:], eye[:])
                nc.scalar.copy(dst[:], ps[:])
                return dst

            def inv32(a_sb, alpha, nm):
                at = transpose32(a_sb, nm)
                xx = spool.tile([DIM, DIM], fp32, name=f"x_{nm}")
                xt = spool.tile([DIM, DIM], fp32, name=f"xt_{nm}")
                nc.vector.tensor_scalar(xx[:], eye[:], alpha, None, op0=OP.mult)
                nc.vector.tensor_scalar(xt[:], eye[:], alpha, None, op0=OP.mult)
                for it in range(NEWTON_ITERS):
                    t1 = qpool.tile([DIM, DIM], fp32, name=f"nt_{nm}_{it}", tag="ps")
                    nc.tensor.matmul(t1[:], at[:], xx[:], start=True, stop=True)
                    w = spool.tile([DIM, DIM], fp32, name=f"w_{nm}_{it}", tag="w")
                    nc.vector.scalar_tensor_tensor(
                        w[:], t1[:], -1.0, eye2[:], op0=OP.mult, op1=OP.add)
                    x2 = qpool.tile([DIM, DIM], fp32, name=f"x2_{nm}_{it}", tag="ps2")
                    xt2 = qpool.tile([DIM, DIM], fp32, name=f"xt2_{nm}_{it}", tag="ps3")
                    nc.tensor.matmul(x2[:], xt[:], w[:], start=True, stop=True)
                    nc.tensor.matmul(xt2[:], w[:], xt[:], start=True, stop=True)
                    nc.scalar.copy(xx[:], x2[:])
                    nc.scalar.copy(xt[:], xt2[:])
                return xx, xt

            # ---- phase A: sigma = inv(stab(B) @ inv(stab(C)) + I) ----
            b_sb = spool.tile([DIM, DIM], fp32, name="b_sb")
            c_sb = spool.tile([DIM, DIM], fp32, name="c_sb")
            nc.sync.dma_start(b_sb[:], b_src)
            nc.sync.dma_start(c_sb[:], c_src)
            bs = stabilize(b_sb, "b")
            cs_ = stabilize(c_sb, "c")
            invc, _ = inv32(cs_, ALPHA_C, "c")
            bt = transpose32(bs, "bt")
            lamp = qpool.tile([DIM, DIM], fp32, name="lamp", tag="ps")
            nc.tensor.matmul(lamp[:], bt[:], invc[:], start=True, stop=True)
            lam = spool.tile([DIM, DIM], fp32, name="lam")
            nc.vector.tensor_add(lam[:], lamp[:], eye[:])
            sigma, _ = inv32(lam, ALPHA_LAM, "s")

            sig_dram = dpool.tile([DIM, DIM], fp32, name="sig_dram")
            nc.sync.dma_start(sig_dram[:], sigma[:])
            sig_rep = tpool.tile([P, DIM * DIM], fp32, name="sig_rep")
            src = sig_dram[:].rearrange("a b -> (a b)").unsqueeze(0) \
                             .broadcast_to([P, DIM * DIM])
            nc.sync.dma_start(sig_rep[:], src)

            # ---- phase B: bootstrap (steps 0..9, one prefix path/partition)
            zb = spool.tile([P, BOOT], fp32, name="zb_sb")
            nc.sync.dma_start(zb[:], zb_src)
            T = tpool.tile([P, DIM * DIM], fp32, name="T_boot")
            nc.vector.tensor_copy(T[:], sig_rep[:])
            Tm = T.rearrange("p (i j) -> p i j", j=DIM)
            Pv = spool.tile([P, BOOT], fp32, name="Pv_boot")
            for k in range(BOOT):
                n = DIM - 1 - k
                nc.vector.tensor_sub(Pv[:, k:k + 1],
                                     T[:, 33 * k:33 * k + 1], zb[:, k:k + 1])
                rv = rpool.tile([P, 1], fp32, name=f"rvb_{k}", tag="rv")
                nc.vector.reciprocal(rv[:], Pv[:, k:k + 1])
                csb_ = rpool.tile([P, n], fp32, name=f"csb_{k}", tag="cs")
                nc.vector.tensor_scalar(csb_[:], Tm[:, k + 1:, k], rv[:, 0:1],
                                        None, op0=OP.mult)
                tvb = d2pool.tile([P, GT * (D2 - 1) * (D2 - 1)], fp32,
                                  name=f"tvb_{k}", tag="tv")[:, :n * n]
                tvv = tvb.rearrange("p (i j) -> p i j", j=n)
                nc.vector.tensor_mul(
                    tvv,
                    csb_[:].unsqueeze(2).broadcast_to([P, n, n]),
                    Tm[:, k:k + 1, k + 1:].broadcast_to([P, n, n]))
                nc.vector.tensor_sub(Tm[:, k + 1:, k + 1:],
                                     Tm[:, k + 1:, k + 1:], tvv)
            d2b = spool.tile([P, BOOT], fp32, name="d2_boot")
            nc.scalar.activation(d2b[:], Pv[:], AF.Square)
            lnb = spool.tile([P, BOOT], fp32, name="ln_boot")
            nc.scalar.activation(lnb[:], d2b[:], AF.Ln)
            bp = spool.tile([P, 1], fp32, name="bp_boot")
            nc.vector.tensor_reduce(bp[:], lnb[:], axis=AX.X, op=OP.add)

            import os as _os2
            _ab = bool(_os2.environ.get("V2_PHASE_AB"))

            # ---- phase C: doubling levels 10..14 ----
            n10 = DIM - BOOT                      # 22
            tabX = tpool.tile([P, 16 * 18 * 18], fp32, name="tabX")
            tabY = tpool.tile([P, SLOTS * ROW], fp32, name="tabY")
            if not _ab:
                nc.vector.tensor_copy(
                    tabX[:, :n10 * n10].rearrange("p (i j) -> p i j", j=n10),
                    Tm[:, BOOT:, BOOT:])
                par = spool.tile([P, 1], fp32, name="par10")
                nc.vector.tensor_copy(par[:], bp[:])

            for k in ([] if _ab else range(BOOT, KPRE)):
                S = 1 << (k - BOOT)
                n = DIM - k
                n2 = n - 1
                last = (k == KPRE - 1)
                stride = ROW if last else n2 * n2
                src_buf = tabX if (k - BOOT) % 2 == 0 else tabY
                dst_buf = tabY if (k - BOOT) % 2 == 0 else tabX
                tab = src_buf[:, :S * n * n]
                tabB = dst_buf[:, :2 * S * stride]
                TmA = tab.rearrange("p (s i j) -> p s i j", i=n, j=n)
                TmB = tabB.rearrange("p (s f) -> p s f", f=stride)
                parB = spool.tile([P, 2 * S], fp32, name=f"par{k + 1}") \
                    if not last else None
                PvL = spool.tile([P, 2 * S], fp32, name=f"PvL{k}")
                nc.vector.tensor_copy(PvL[:, 0::2], TmA[:, :, 0, 0])
                nc.vector.tensor_scalar(PvL[:, 1::2], TmA[:, :, 0, 0], -1.0,
                                        None, op0=OP.add)
                rv = rpool.tile([P, 2 * S], fp32, name=f"rvl_{k}", tag="rv")
                nc.vector.reciprocal(rv[:], PvL[:])
                d2l = d2pool.tile([P, 2 * S], fp32, name=f"d2l_{k}", tag="d2")
                nc.scalar.activation(d2l[:], PvL[:], AF.Square)
                lnl = d2pool.tile([P, 2 * S], fp32, name=f"lnl_{k}", tag="ln")
                nc.scalar.activation(lnl[:], d2l[:], AF.Ln)
                for z in (0, 1):
                    pdst = TmB[:, z::2, n2 * n2] if last else parB[:, z::2]
                    nc.vector.tensor_add(pdst, par[:], lnl[:, z::2])
                    csl = rpool.tile([P, S * n2], fp32, name=f"csl_{k}_{z}",
                                     tag="cs")
                    cs3 = csl.rearrange("p (s i) -> p s i", i=n2)
                    nc.vector.tensor_mul(
                        cs3, TmA[:, :, 1:, 0],
                        rv[:, z::2].unsqueeze(2).broadcast_to([P, S, n2]))
                    tvl = d2pool.tile([P, S * n2 * n2], fp32,
                                      name=f"tvl_{k}_{z}", tag="tv")
                    tv4 = tvl.rearrange("p (s i j) -> p s i j", i=n2, j=n2)
                    nc.vector.tensor_mul(
                        tv4,
                        cs3[:, :, :].unsqueeze(3).broadcast_to([P, S, n2, n2]),
                        TmA[:, :, 0:1, 1:].broadcast_to([P, S, n2, n2]))
                    child = TmB[:, z::2, 0:n2 * n2] \
                        .rearrange("p s (i j) -> p s i j", j=n2)
                    nc.vector.tensor_sub(child, TmA[:, :, 1:, 1:], tv4)
                par = parB

            # ---- phase D: table to DRAM ----
            table = tdpool.tile([NNODES, ROW], fp32, name="table_dram")
            if not _ab:
                nc.sync.dma_start(
                    table[:].rearrange("(p s) f -> p (s f)", p=P),
                    tabY[:, :SLOTS * ROW])

            # ---- phase E: gather + per-sample elimination ----
            idx = spool.tile([P, NTILES], i32, name="idx_sb")
            nc.sync.dma_start(idx[:], idx_src)
            zf = gpool.tile([P, NTILES * D2], fp32, name="zf")
            nc.sync.dma_start(zf[:], zf_src)
            zf3 = zf[:].rearrange("p (t d) -> p t d", d=D2)

            out = spool.tile([P, NTILES], fp32, name="out_sb")

            import os as _os
            _noind = bool(_os.environ.get("V2_NO_INDIRECT"))
            mgs = []
            for g in ([] if _ab else range(NGROUPS)):
                mg = gpool.tile([P, GT * ROW], fp32, name=f"m_{g}")
                mgs.append(mg)
                if _noind:
                    nc.sync.dma_start(
                        mg[:].rearrange("p (t f) -> p t f", f=ROW),
                        table[:].rearrange("(p s) f -> p s f", p=P)[:, 0:GT, :])
                    continue
                for t in range(GT):
                    nc.gpsimd.indirect_dma_start(
                        out=mg[:, t * ROW:(t + 1) * ROW],
                        out_offset=None,
                        in_=table[:],
                        in_offset=bass.IndirectOffsetOnAxis(
                            ap=idx[:, g * GT + t:g * GT + t + 1], axis=0),
                    )

            for g in ([] if _ab else range(NGROUPS)):
                mg = mgs[g]
                m3 = mg.rearrange("p (t f) -> p t f", f=ROW)
                mv = m3[:, :, 0:D2 * D2].rearrange("p t (i j) -> p t i j", j=D2)
                dview = m3[:, :, 0:D2 * D2:D2 + 1]          # [P, GT, 17]
                nc.vector.tensor_sub(dview, dview,
                                     zf3[:, g * GT:(g + 1) * GT, :])
                rg = rpool.tile([P, GT], fp32, name=f"rg_{g}", tag="rg")
                csg = rpool.tile([P, GT * (D2 - 1)], fp32, name=f"cse_{g}",
                                 tag="cse")
                for j in range(D2 - 1):
                    n = D2 - 1 - j
                    nc.vector.reciprocal(rg[:], mv[:, :, j, j])
                    csv = csg.rearrange("p (t i) -> p t i", i=D2 - 1)[:, :, :n]
                    nc.vector.tensor_mul(
                        csv, mv[:, :, j + 1:, j],
                        rg[:].unsqueeze(2).broadcast_to([P, GT, n]))
                    tt = d2pool.tile([P, GT * n * n], fp32, name=f"te_{g}_{j}",
                                     tag="tv")
                    tv4 = tt.rearrange("p (t i j) -> p t i j", i=n, j=n)
                    nc.vector.tensor_mul(
                        tv4,
                        csv.unsqueeze(3).broadcast_to([P, GT, n, n]),
                        mv[:, :, j:j + 1, j + 1:].broadcast_to([P, GT, n, n]))
                    nc.vector.tensor_sub(mv[:, :, j + 1:, j + 1:],
                                         mv[:, :, j + 1:, j + 1:], tv4)
                d2e = d2pool.tile([P, GT * D2], fp32, name=f"d2e_{g}", tag="d2")
                nc.scalar.activation(d2e[:], dview, AF.Square)
                lne = d2pool.tile([P, GT * D2], fp32, name=f"lne_{g}", tag="ln")
                nc.scalar.activation(lne[:], d2e[:], AF.Ln)
                red = rpool.tile([P, GT], fp32, name=f"red_{g}", tag="red")
                nc.vector.tensor_reduce(
                    red[:].unsqueeze(2),
                    lne[:].rearrange("p (t d) -> p t d", d=D2),
                    axis=AX.X, op=OP.add)
                nc.vector.tensor_add(out[:, g * GT:(g + 1) * GT], red[:],
                                     m3[:, :, D2 * D2])

            if _ab:
                nc.vector.memset(out[:], 0.0)
                nc.vector.tensor_add(out[:, 0:1], out[:, 0:1], bp[:])
            nc.sync.dma_start(out_d[:], out[:])

    return nc


def _get():
    if "nc" not in _cache:
        _cache["nc"] = _build()
    return _cache["nc"]


def _legalize_bir(bir_json: bytes) -> bytes:
    """Walrus allows only ONE embedded sem wait per instruction; split extra
    waits into standalone EventSemaphore instructions."""
    import json as _json
    j = _json.loads(bir_json)
    n_split = 0
    for fn in j.get("functions", []):
        for blk in fn.get("blocks", []):
            out = []
            for inst in blk.get("instructions", []):
                si = inst.get("sync_info") or {}
                waits = si.get("on_wait") or []
                if len(waits) > 1:
                    for wi, w in enumerate(waits[:-1]):
                        out.append({
                            "debug": 0,
                            "engine": inst.get("engine", "Unassigned"),
                            "ins": [], "outs": [],
                            "name": f"{inst.get('name','I')}-w{wi}",
                            "opcode": "EventSemaphore",
                            "sync_info": {"on_wait": [w], "on_update": []},
                        })
                        n_split += 1
                    si = dict(si)
                    si["on_wait"] = [waits[-1]]
                    inst = dict(inst)
                    inst["sync_info"] = si
                out.append(inst)
            blk["instructions"] = out
    if n_split:
        print(f"[legalize] split {n_split} extra sem waits")
    return _json.dumps(j).encode()


_patched = False


def _install_patch():
    global _patched
    if _patched:
        return
    import concourse.bass_utils as bu
    import concourse.bass2jax as b2j
    orig = bu.compile_bir_kernel

    def patched(bir_json, tmpdir, neff_name="file.neff"):
        return orig(_legalize_bir(bir_json), tmpdir, neff_name)

    bu.compile_bir_kernel = patched
    b2j.compile_bir_kernel = patched
    _patched = True


def _preprocess(x, B, C):
    """Sort samples by 15-bit prefix, route to cores by top 3 bits, pad,
    and pack each core's inputs into one int32 blob."""
    x = np.ascontiguousarray(np.asarray(x, dtype=np.int32))
    B = np.asarray(B, dtype=np.float32)
    C = np.asarray(C, dtype=np.float32)
    eye = np.eye(DIM, dtype=np.float32)
    z = (1 - x).astype(np.int64)
    prefix = np.zeros(len(x), dtype=np.int64)
    for k in range(KPRE):
        prefix = (prefix << 1) | z[:, k]
    core = (prefix >> (KPRE - 3)).astype(np.int64)
    row = (prefix & (NNODES - 1)).astype(np.int32)

    blobs, counts = [], []
    for c in range(NCORES):
        sel = np.nonzero(core == c)[0]
        ncs = len(sel)
        assert ncs <= CAP, f"core {c} overflow: {ncs} > {CAP}"
        xc = np.empty((CAP, DIM), dtype=np.int32)
        rc = np.empty(CAP, dtype=np.int32)
        xc[:ncs] = x[sel]
        rc[:ncs] = row[sel]
        if ncs < CAP:
            xc[ncs:] = xc[0]
            rc[ncs:] = rc[0]
        idx_pt = rc.reshape(NTILES, P).T   # out[p, t] <-> shard row t*P+p
        node = c * P + np.arange(P)
        zb = np.empty((P, BOOT), dtype=np.float32)
        for k in range(BOOT):
            zb[:, k] = (node >> (BOOT - 1 - k)) & 1

        zc = (1 - xc[:, KPRE:DIM]).astype(np.float32)      # [CAP, 17]
        zf = np.ascontiguousarray(
            zc.reshape(NTILES, P, D2).transpose(1, 0, 2)).reshape(P, -1)
        blob = np.empty((BLOB_ROWS, DIM), dtype=np.int32)
        blob[0:ZF_ROWS] = zf.reshape(-1).view(np.int32).reshape(ZF_ROWS, DIM)
        blob[ROW_B:ROW_B + 32] = B.view(np.int32)
        blob[ROW_C:ROW_C + 32] = C.view(np.int32)
        blob[ROW_EYE:ROW_EYE + 32] = eye.view(np.int32)
        blob[ROW_ZB:ROW_ZB + 40] = zb.reshape(-1).view(np.int32).reshape(40, DIM)
        blob[ROW_IDX:ROW_IDX + 272] = \
            np.ascontiguousarray(idx_pt).reshape(-1).reshape(272, DIM)
        blobs.append(blob)
        counts.append(ncs)
    return blobs, counts


def _run(x, B, C, ncores=NCORES, trace=False):
    from concourse.bass_utils import run_bass_kernel_spmd
    _install_patch()

    blobs, counts = _preprocess(x, B, C)
    nc = _get()
    in_maps = [{"blob": blobs[c]} for c in range(ncores)]
    res = run_bass_kernel_spmd(nc, in_maps, core_ids=list(range(ncores)),
                               trace=trace)
    return res, counts


def _reduce(res_results, counts):
    total = 0.0
    for c, r in enumerate(res_results):
        o = r["out"]                       # [P, NTILES]
        ncs = counts[c]
        vals = o.T.reshape(-1)             # slot s = t*P+p -> o[p, t]
        total += vals[:ncs].astype(np.float64).sum()
    return np.float32(0.5 * total / BATCH)


def kernel(x, B, C):
    res, counts = _run(x, B, C)
    return _reduce(res.results, counts)
